# revision 29
# baseline (speedup 1.0000x reference)
# Trainium2 Bass kernel for nn_Attention_65609920413963 (sparse block-masked
# attention), v7: paired-head ST/OT tiles, engine rebalance, strided-AP
# copies, multi-queue DMA, 3-stage cross-batch software pipeline, legacy
# tile scheduler (2x faster than ASAP on this program, on HW and in sim).
#
# Math structure (verified against reference numerics):
#   L_b = n1[b]*n2[b].  Rows >= L_b are fully masked -> softmax over K row
#   only; host computes those rows (cheap).  Rows f < L_b see keys k < L_b,
#   with multiplicative bias exp(K[b,f,k]) folded into the post-exp multiply
#   (ek), zeros marking masked keys.
#
# Sharding: every core runs the SAME program on head-pair j = core_id of ALL
# batches (channels [128j, 128j+128) of Q/K/V).  Per-batch work is exact:
# queries F_b = L_b, keys padded to K_b = ceil(L_b/128) tiles.  Final
# projection partials (one per core, 128-channel contraction) are summed on
# the host in f32.
#
# Key engine layout (per (chunk, key-tile)):
#   PE : ST_e -> pair bank 0, ST_o -> pair bank 1  (one 2-bank PSUM tile)
#   ACT: one exp over a strided [vk, 2, ql] AP covering both heads -> et
#   DVE/Pool: et * ek -> pt halves (mask+bias application)
#   PE : OT_e/OT_o accumulation; vp carries a 64-wide ones block so OT rows
#        64:128 (even) / 0:64 (odd) come out as the softmax denominator
#   DVE: 2 reciprocals + 2 multiplies -> otn (bf16) per chunk
#   PE : bf16 projection; ACT/DVE copy to ys; SP DMAs y out.
import os as _os

# The legacy tile scheduler both schedules this program correctly and beats
# the v2 ASAP scheduler by ~2x on it (measured on hardware *and* in CoreSim),
# so the build below defaults to it (TILE_SCHEDULER unset = legacy).

import numpy as np

B, N, C = 4, 1024, 1024
H, Dh = 16, 64
NCC = C // 128  # 8 contraction chunks

_CACHE = {}


def _shapes(Ls):
    F = [int(l) for l in Ls]                      # exact query rows
    K = [-(-f // 128) for f in F]                 # key tiles
    XW = [k * 128 for k in K]                     # padded key/x width
    R = [-(-f // 128) for f in F]                 # row tiles for proj
    xoff = np.cumsum([0] + F).tolist()            # offsets into xt packing
    yoff = np.cumsum([0] + F).tolist()            # offsets into y/qt/otn
    ekoff = np.cumsum([0] + [K[b] * F[b] for b in range(len(F))]).tolist()
    koff = np.cumsum([0] + K).tolist()            # vp key-tile offsets
    return F, K, XW, R, xoff, yoff, ekoff, koff


def _chunks(total, cap=512):
    n = -(-total // cap)
    base = -(-total // n)
    out = []
    off = 0
    while off < total:
        w = min(base, total - off)
        out.append((off, w))
        off += w
    return out


def _build_program(key, reps=1):
    import os
    # legacy scheduler by default (faster than ASAP here; handles the For_i
    # timing loop too).  FORCE_ASAP is kept as an escape hatch.
    if os.environ.get("FORCE_ASAP") and reps == 1:
        os.environ["TILE_SCHEDULER"] = "asap"
    else:
        os.environ.pop("TILE_SCHEDULER", None)
    import concourse.bacc as bacc
    import concourse.bass as bass
    import concourse.mybir as mybir
    import concourse.tile as tile
    import contextlib

    Ls = list(key)
    F, K, XW, R, xoff, yoff, ekoff, koff = _shapes(Ls)
    NB = len(F)
    FT = yoff[-1]          # total query rows (== xt width, valid rows only)
    EKW = ekoff[-1]        # total ek width
    KT = koff[-1]          # total key tiles

    F32 = mybir.dt.float32
    F16 = mybir.dt.float16
    BF16 = mybir.dt.bfloat16
    F8 = mybir.dt.float8e4

    nc = bacc.Bacc("TRN2", target_bir_lowering=False, debug=False)

    xt_d = nc.dram_tensor("xt", [128, NCC, FT], F16, kind="ExternalInput")
    wq_d = nc.dram_tensor("wq", [128, NCC, 128], F16, kind="ExternalInput")
    wk_d = nc.dram_tensor("wk", [128, NCC, 128], F16, kind="ExternalInput")
    wv_d = nc.dram_tensor("wv", [128, NCC, 128], F16, kind="ExternalInput")
    pw_d = nc.dram_tensor("pw", [128, C], BF16, kind="ExternalInput")
    ek_d = nc.dram_tensor("ek", [128, EKW], BF16, kind="ExternalInput")
    y_d = nc.dram_tensor("y", [FT, C], BF16, kind="ExternalOutput")

    def vkey(b, kt):
        return min(128, F[b] - kt * 128)   # valid keys in tile kt

    # process order: big batches first (deep weave), smallest key-count last
    # (short un-weavable tail)
    border = sorted(range(NB), key=lambda b: (-K[b], -F[b]))

    with tile.TileContext(nc) as tc:
        with (
            tc.For_i(0, reps, 1) if reps > 1 else contextlib.nullcontext(),
            tc.tile_pool(name="yspool", bufs=3) as yspool,
            tc.tile_pool(name="work", bufs=3) as work,
            tc.tile_pool(name="ptpool", bufs=3) as ptpool,
            tc.tile_pool(name="singles", bufs=1) as singles,
            tc.tile_pool(name="psA", bufs=2, space="PSUM") as psA,
            tc.tile_pool(name="psST", bufs=2, space="PSUM") as psST,
            tc.tile_pool(name="psOT", bufs=1, space="PSUM") as psOT,
        ):
            # ---- resident SBUF tensors (xt last: address-layout sensitive) --
            nbias_sb = singles.tile([128, 1], F32, tag="nbias")
            qt_sb = singles.tile([128, FT], F16, tag="qt")
            kt_sb = singles.tile([128, FT], F16, tag="kt")
            otn_sb = singles.tile([128, FT], BF16, tag="otn")
            vp_sb = singles.tile([128, KT, 4, 64], BF16, tag="vp")
            ek_sb = singles.tile([128, EKW], BF16, tag="ek")
            wq_sb = singles.tile([128, NCC, 128], F16, tag="wq")
            wk_sb = singles.tile([128, NCC, 128], F16, tag="wk")
            wv_sb = singles.tile([128, NCC, 128], F16, tag="wv")
            pw_sb = singles.tile([128, C], BF16, tag="pw")
            xt_sb = singles.tile([128, NCC, FT], F16, tag="xt")

            nc.vector.memset(nbias_sb, -44.0)

            # ---- input DMAs ------------------------------------------------
            # SP carries the critical path + late batches; Pool (SWDGE) takes
            # the second batch early, before its multiply stream starts; ACT
            # only pw (tiny).  DVE stays DMA-free.
            def xdma(b, eng):
                eng.dma_start(
                    out=xt_sb[:, :, xoff[b] : xoff[b] + F[b]],
                    in_=xt_d.ap()[:, :, xoff[b] : xoff[b] + F[b]],
                )

            def ekdma(b, eng):
                eng.dma_start(
                    out=ek_sb[:, ekoff[b] : ekoff[b + 1]],
                    in_=ek_d.ap()[:, ekoff[b] : ekoff[b + 1]],
                )

            o0, o1, o2, o3 = border
            nc.sync.dma_start(out=wq_sb, in_=wq_d.ap())
            # first batch's x split in chunks so its first QKV matmuls can
            # start as soon as the first chunk lands
            for qo, ql in _chunks(F[o0]):
                nc.sync.dma_start(
                    out=xt_sb[:, :, xoff[o0] + qo : xoff[o0] + qo + ql],
                    in_=xt_d.ap()[:, :, xoff[o0] + qo : xoff[o0] + qo + ql],
                )
            nc.sync.dma_start(out=wk_sb, in_=wk_d.ap())
            nc.sync.dma_start(out=wv_sb, in_=wv_d.ap())
            xdma(o2, eng=nc.sync)
            ekdma(o2, eng=nc.sync)
            xdma(o3, eng=nc.sync)
            ekdma(o3, eng=nc.sync)
            ekdma(o0, eng=nc.gpsimd)
            xdma(o1, eng=nc.gpsimd)
            ekdma(o1, eng=nc.gpsimd)
            nc.scalar.dma_start(out=pw_sb, in_=pw_d.ap())

            # ---- phase thunks ---------------------------------------------
            DMAONLY = bool(os.environ.get("V7DMAONLY"))
            NOY = bool(os.environ.get("V7NOY"))
            EXPSPLIT = bool(os.environ.get("V7EXPSPLIT"))
            VPSIMPLE = bool(os.environ.get("V7VPSIMPLE"))
            NOPOOLMUL = bool(os.environ.get("V7NOPOOLMUL"))
            ekmul_i = [0]  # DVE/Pool split counter for the et*ek multiplies

            def qkv_thunks(b):
                ts = []
                for w_sb, t_sb in ((wq_sb, qt_sb), (wk_sb, kt_sb)):
                    for qo, ql in _chunks(F[b]):
                        def t(w_sb=w_sb, t_sb=t_sb, qo=qo, ql=ql, b=b):
                            if DMAONLY:
                                return
                            ps = psA.tile([128, 512], F32, tag="ps")
                            for cc in range(NCC):
                                nc.tensor.matmul(
                                    ps[:, 0:ql],
                                    w_sb[:, cc],
                                    xt_sb[:, cc, xoff[b] + qo : xoff[b] + qo + ql],
                                    start=(cc == 0),
                                    stop=(cc == NCC - 1),
                                )
                            nc.scalar.copy(
                                out=t_sb[:, xoff[b] + qo : xoff[b] + qo + ql],
                                in_=ps[:, 0:ql],
                            )
                        ts.append(t)
                # V: pack up to 4 full key tiles per PSUM bank; the final
                # partial tile (vk < 128) gets its own allocation so the
                # group copy never reads uninitialized rows.
                groups = []
                nfull = K[b] if F[b] % 128 == 0 else K[b] - 1
                kt0 = 0
                while kt0 < nfull:
                    gn = min(4, nfull - kt0)
                    groups.append((kt0, gn))
                    kt0 += gn
                if nfull < K[b]:
                    groups.append((nfull, 1))
                for kt0, gn in groups:
                    def t(kt0=kt0, gn=gn, b=b):
                        if DMAONLY:
                            return
                        vkl = vkey(b, kt0 + gn - 1)   # last tile may be short
                        ps = psA.tile([128, 4, 2, 64], F32, tag="ps", name="vps")
                        for g in range(gn):
                            kt = kt0 + g
                            vk = vkey(b, kt)
                            for cc in range(NCC):
                                nc.tensor.matmul(
                                    ps[0:vk, g, :, :],
                                    xt_sb[:, cc, xoff[b] + kt * 128 : xoff[b] + kt * 128 + vk],
                                    wv_sb[:, cc],
                                    start=(cc == 0),
                                    stop=(cc == NCC - 1),
                                    skip_group_check=True,
                                )
                        kk = koff[b] + kt0
                        # vp blocks per key tile: [ch_e | ones | ones | ch_o];
                        # OT_e reads blocks 0:2 = [channels | denom ones],
                        # OT_o reads 2:4.  One strided copy per group fills
                        # all channel blocks.
                        if VPSIMPLE:
                            for g in range(gn):
                                vk = vkey(b, kt0 + g)
                                nc.vector.tensor_copy(
                                    out=vp_sb[0:vk, kk + g, 0, :],
                                    in_=ps[0:vk, g, 0, :],
                                )
                                nc.vector.tensor_copy(
                                    out=vp_sb[0:vk, kk + g, 3, :],
                                    in_=ps[0:vk, g, 1, :],
                                )
                                nc.gpsimd.memset(vp_sb[:, kk + g, 1:3, :], 1.0)
                        else:
                            if gn == 1:
                                nc.vector.tensor_copy(
                                    out=vp_sb[0:vkl, kk, 0:4:3, :],
                                    in_=ps[0:vkl, 0, :, :],
                                )
                            else:
                                nc.vector.tensor_copy(
                                    out=vp_sb[:, kk : kk + gn, 0:4:3, :],
                                    in_=ps[:, 0:gn, :, :],
                                )
                            nc.gpsimd.memset(vp_sb[:, kk : kk + gn, 1:3, :], 1.0)
                    ts.append(t)
                return ts

            def attn_thunks(b):
                ts = []
                for qo, ql in _chunks(F[b]):
                    ot_pair = []
                    st_q = []

                    def emit_st(kt, qo=qo, ql=ql, b=b):
                        if DMAONLY:
                            return
                        vk = vkey(b, kt)
                        # both heads' logits in one 2-bank tile: head h in
                        # bank h, so a single strided [vk, 2, ql] AP covers
                        # both for the exp without touching the bank gaps
                        stp = psST.tile([128, 2, 512], F32, tag="st")
                        for lo, hi, par in ((0, 64, 0), (64, 128, 1)):
                            nc.tensor.matmul(
                                stp[0:vk, par, 0:ql],
                                kt_sb[lo:hi, xoff[b] + kt * 128 : xoff[b] + kt * 128 + vk],
                                qt_sb[lo:hi, yoff[b] + qo : yoff[b] + qo + ql],
                                start=True,
                                stop=True,
                                skip_group_check=True,
                            )
                        st_q.append((kt, stp))

                    def emit_tail(qo=qo, ql=ql, b=b, ot_pair=ot_pair):
                        if DMAONLY:
                            return
                        kt, stp = st_q.pop(0)
                        vk = vkey(b, kt)
                        if kt == 0:
                            ot_pair.append(
                                psOT.tile([128, 2, 512], F32, tag="ot", name="otp")
                            )
                        otp = ot_pair[0]
                        e0 = ekoff[b] + kt * F[b] + qo
                        et = work.tile([128, 2, 512], BF16, tag="et")
                        if EXPSPLIT:
                            for par in (0, 1):
                                nc.scalar.activation(
                                    out=et[0:vk, par, 0:ql],
                                    in_=stp[0:vk, par, 0:ql],
                                    func=mybir.ActivationFunctionType.Exp,
                                    bias=nbias_sb[0:vk, :],
                                )
                        else:
                            nc.scalar.activation(
                                out=et[0:vk, 0:2, 0:ql],
                                in_=stp[0:vk, 0:2, 0:ql],
                                func=mybir.ActivationFunctionType.Exp,
                                bias=nbias_sb[0:vk, :],
                            )
                        pt = ptpool.tile([128, 2, 512], BF16, tag="pt")
                        # 1-partition contractions fault the device; for a
                        # single valid key pad the contraction with a zeroed
                        # second row (vp rows are zeroed too so 0*0 stays 0)
                        vkc = max(vk, 2)
                        if vk < 2:
                            nc.vector.memset(pt[vk:2, :, 0:ql], 0.0)
                            nc.vector.memset(
                                vp_sb[vk:2, koff[b] + kt, :, :], 0.0
                            )
                        for par in (0, 1):
                            # every 4th multiply goes to Pool to relieve DVE
                            eng = nc.vector if NOPOOLMUL else (
                                nc.gpsimd if ekmul_i[0] % 3 == 2 else nc.vector)
                            ekmul_i[0] += 1
                            eng.tensor_mul(
                                pt[0:vk, par, 0:ql],
                                et[0:vk, par, 0:ql],
                                ek_sb[0:vk, e0 : e0 + ql],
                            )
                            nc.tensor.matmul(
                                otp[:, par, 0:ql],
                                vp_sb[0:vkc, koff[b] + kt, 2 * par : 2 * par + 2, :],
                                pt[0:vkc, par, 0:ql],
                                start=(kt == 0),
                                stop=(kt == K[b] - 1),
                            )

                    # ST-lead pipeline: PE runs kt+1's ST matmuls while kt's
                    # exp/mult chain drains
                    for kt in range(K[b]):
                        def t(kt=kt, es=emit_st, etl=emit_tail, kb=K[b]):
                            es(kt)
                            if kt >= 1:
                                etl()
                            if kt == kb - 1:
                                etl()
                        ts.append(t)

                    def t(qo=qo, ql=ql, b=b, ot_pair=ot_pair):
                        if DMAONLY:
                            return
                        otp = ot_pair[0]
                        qa = yoff[b] + qo
                        rb = work.tile([128, 512], BF16, tag="rb")
                        with nc.allow_low_precision(reason="bf16 denom recip"):
                            nc.vector.reciprocal(
                                out=rb[0:64, 0:ql], in_=otp[64:128, 0, 0:ql]
                            )
                            nc.vector.reciprocal(
                                out=rb[64:128, 0:ql], in_=otp[0:64, 1, 0:ql]
                            )
                        nc.vector.tensor_mul(
                            otn_sb[0:64, qa : qa + ql],
                            otp[0:64, 0, 0:ql], rb[0:64, 0:ql],
                        )
                        nc.vector.tensor_mul(
                            otn_sb[64:128, qa : qa + ql],
                            otp[64:128, 1, 0:ql], rb[64:128, 0:ql],
                        )
                    ts.append(t)
                return ts

            def proj_thunks(b):
                ts = []
                pi = [0]
                for rt in range(R[b]):
                    def t(rt=rt, b=b):
                        if DMAONLY or NOY:
                            return
                        rows = min(128, F[b] - rt * 128)
                        r0 = yoff[b] + rt * 128
                        ys = yspool.tile([128, C], BF16, tag="ys")
                        for oc in range(2):
                            pj = psA.tile([128, 512], F32, tag="ps", name="pj")
                            nc.tensor.matmul(
                                pj[0:rows, :],
                                otn_sb[:, r0 : r0 + rows],
                                pw_sb[:, oc * 512 : (oc + 1) * 512],
                                start=True,
                                stop=True,
                            )
                            if pi[0] % 2 == 0:
                                nc.scalar.copy(
                                    out=ys[0:rows, oc * 512 : (oc + 1) * 512],
                                    in_=pj[0:rows, :],
                                )
                            else:
                                nc.vector.tensor_copy(
                                    out=ys[0:rows, oc * 512 : (oc + 1) * 512],
                                    in_=pj[0:rows, :],
                                )
                            pi[0] += 1
                        nc.sync.dma_start(
                            out=y_d.ap()[r0 : r0 + rows, :],
                            in_=ys[0:rows, :],
                        )
                    ts.append(t)
                return ts

            # ---- 3-stage pipelined emission -------------------------------
            # qkv(b_{i+1}) and proj(b_{i-1}) weave into attn(b_i)'s stream.
            A = [qkv_thunks(b) for b in border]
            Bt = [attn_thunks(b) for b in border]
            Ct = [proj_thunks(b) for b in border]

            if os.environ.get("V7SERIAL"):
                for i in range(NB):
                    for t in A[i] + Bt[i] + Ct[i]:
                        t()
            else:
                for t in A[0]:
                    t()
                for i in range(NB):
                    lists = [Bt[i]]
                    if i + 1 < NB:
                        lists.append(A[i + 1])
                    if i - 1 >= 0:
                        lists.append(Ct[i - 1])
                    while any(lists):
                        for ls in lists:
                            if ls:
                                ls.pop(0)()
                for t in Ct[NB - 1]:
                    t()

    nc.compile()
    return nc


def _pad_for(L):
    # retained name for test.py compatibility: returns the program cache key
    return tuple(int(l) for l in L)


def _prep_inputs(key, x, K, n1, n2, qkv_w, qkv_b, proj_w):
    import ml_dtypes

    Ls = list(key)
    F, Kt, XW, R, xoff, yoff, ekoff, koff = _shapes(Ls)
    FT, EKW = yoff[-1], ekoff[-1]
    scale = np.float32(Dh**-0.5)
    assert not np.any(qkv_b), "nonzero qkv_b not supported by this kernel"
    bf16 = ml_dtypes.bfloat16
    f8 = ml_dtypes.float8_e4m3fn

    # xt: [128, FT, 8] fp16 (valid rows only, batches concatenated; per-batch
    # slices are fully contiguous per partition for single-descriptor DMAs)
    X_all = np.empty((FT, C), dtype=np.float16)
    for b in range(B):
        X_all[xoff[b] : xoff[b] + F[b]] = x[b, : F[b]]
    xt = np.ascontiguousarray(
        X_all.T.reshape(NCC, 128, FT).transpose(1, 0, 2)
    )

    # ek: [128, EKW] bf16 multiplicative exp(K), zeros on padded/masked keys
    ekp = np.zeros((128, EKW), dtype=bf16)
    for b in range(B):
        E = np.zeros((XW[b], F[b]), dtype=np.float32)
        E[: F[b], :] = np.exp(K[b, : F[b], : F[b]].astype(np.float32)).T
        ekp[:, ekoff[b] : ekoff[b + 1]] = (
            E.reshape(Kt[b], 128, F[b]).transpose(1, 0, 2).reshape(128, -1)
        )

    def wslice(w, j):
        # w rows [128j:128j+128] of [C, C]; -> [128 p(cc), 8 cc, 128 m]
        ws = np.ascontiguousarray(
            w[128 * j : 128 * (j + 1), :].T.reshape(NCC, 128, 128).transpose(1, 0, 2)
        )
        return ws

    in_maps = []
    for j in range(8):
        wq = wslice(qkv_w[0 * C : 1 * C] * scale, j).astype(np.float16)
        wk = wslice(qkv_w[1 * C : 2 * C], j).astype(np.float16)
        wv = wslice(qkv_w[2 * C : 3 * C], j).astype(np.float16)
        pw = np.ascontiguousarray(
            proj_w[:, 128 * j : 128 * (j + 1)].T
        ).astype(bf16)
        in_maps.append(
            {"xt": xt, "wq": wq, "wk": wk, "wv": wv, "pw": pw, "ek": ekp}
        )
    L = np.asarray(Ls, dtype=np.int32)
    return in_maps, L


def run_device(inputs, trace=False):
    """Compile (cached), run on 8 cores, return (BassKernelResults, L)."""
    from concourse import bass_utils

    x = np.asarray(inputs["x"], dtype=np.float32)
    K = np.asarray(inputs["K"], dtype=np.float32)
    n1 = np.asarray(inputs["n1"])
    n2 = np.asarray(inputs["n2"])
    L = (n1.astype(np.int64) * n2.astype(np.int64)).astype(np.int32)
    key = _pad_for(L)
    if ("nc", key) not in _CACHE:
        _CACHE[("nc", key)] = _build_program(key)
    nc = _CACHE[("nc", key)]

    in_maps, L = _prep_inputs(
        key, x, K, n1, n2,
        np.asarray(inputs["qkv_w"], dtype=np.float32),
        np.asarray(inputs["qkv_b"], dtype=np.float32),
        np.asarray(inputs["proj_w"], dtype=np.float32),
    )
    res = bass_utils.run_bass_kernel_spmd(
        nc, in_maps, core_ids=list(range(8)), trace=trace
    )
    return res, L


def kernel(**inputs):
    x = np.asarray(inputs["x"], dtype=np.float32)
    qkv_w = np.asarray(inputs["qkv_w"], dtype=np.float32)
    qkv_b = np.asarray(inputs["qkv_b"], dtype=np.float32)
    proj_w = np.asarray(inputs["proj_w"], dtype=np.float32)
    proj_b = np.asarray(inputs["proj_b"], dtype=np.float32)

    res, L = run_device(inputs)
    Fs, _, _, _, _, yoff, _, _ = _shapes(L)

    ysum = np.zeros((yoff[-1], C), dtype=np.float32)
    for r in res.results:
        ysum += np.asarray(r["y"], dtype=np.float32)
    ysum += proj_b

    out = np.empty((B, N, C), dtype=np.float32)
    for b in range(B):
        Lb = int(L[b])
        out[b, :Lb] = ysum[yoff[b] : yoff[b] + Lb]
        # fully-masked rows: exactly uniform softmax -> mean of V
        vbar = x[b].mean(axis=0) @ qkv_w[2 * C : 3 * C, :].T + qkv_b[2 * C : 3 * C]
        out[b, Lb:] = vbar @ proj_w.T + proj_b
    return out


# revision 30
# speedup vs baseline: 1.0376x; 1.0376x over previous
# Trainium2 Bass kernel for nn_Attention_65609920413963 (sparse block-masked
# attention), v7: paired-head ST/OT tiles, engine rebalance, strided-AP
# copies, multi-queue DMA, 3-stage cross-batch software pipeline, legacy
# tile scheduler (2x faster than ASAP on this program, on HW and in sim).
#
# Math structure (verified against reference numerics):
#   L_b = n1[b]*n2[b].  Rows >= L_b are fully masked -> softmax over K row
#   only; host computes those rows (cheap).  Rows f < L_b see keys k < L_b,
#   with multiplicative bias exp(K[b,f,k]) folded into the post-exp multiply
#   (ek), zeros marking masked keys.
#
# Sharding: every core runs the SAME program on head-pair j = core_id of ALL
# batches (channels [128j, 128j+128) of Q/K/V).  Per-batch work is exact:
# queries F_b = L_b, keys padded to K_b = ceil(L_b/128) tiles.  Final
# projection partials (one per core, 128-channel contraction) are summed on
# the host in f32.
#
# Key engine layout (per (chunk, key-tile)):
#   PE : ST_e -> pair bank 0, ST_o -> pair bank 1  (one 2-bank PSUM tile)
#   ACT: one exp over a strided [vk, 2, ql] AP covering both heads -> et
#   DVE/Pool: et * ek -> pt halves (mask+bias application)
#   PE : OT_e/OT_o accumulation; vp carries a 64-wide ones block so OT rows
#        64:128 (even) / 0:64 (odd) come out as the softmax denominator
#   DVE: 2 reciprocals + 2 multiplies -> otn (bf16) per chunk
#   PE : bf16 projection; ACT/DVE copy to ys; SP DMAs y out.
import os as _os

# The legacy tile scheduler both schedules this program correctly and beats
# the v2 ASAP scheduler by ~2x on it (measured on hardware *and* in CoreSim),
# so the build below defaults to it (TILE_SCHEDULER unset = legacy).

import numpy as np

B, N, C = 4, 1024, 1024
H, Dh = 16, 64
NCC = C // 128  # 8 contraction chunks

_CACHE = {}


def _shapes(Ls):
    F = [int(l) for l in Ls]                      # exact query rows
    K = [-(-f // 128) for f in F]                 # key tiles
    XW = [k * 128 for k in K]                     # padded key/x width
    R = [-(-f // 128) for f in F]                 # row tiles for proj
    xoff = np.cumsum([0] + F).tolist()            # offsets into xt packing
    yoff = np.cumsum([0] + F).tolist()            # offsets into y/qt/otn
    ekoff = np.cumsum([0] + [K[b] * F[b] for b in range(len(F))]).tolist()
    koff = np.cumsum([0] + K).tolist()            # vp key-tile offsets
    return F, K, XW, R, xoff, yoff, ekoff, koff


def _chunks(total, cap=512):
    n = -(-total // cap)
    base = -(-total // n)
    out = []
    off = 0
    while off < total:
        w = min(base, total - off)
        out.append((off, w))
        off += w
    return out


def _build_program(key, reps=1):
    import os
    # legacy scheduler by default (faster than ASAP here; handles the For_i
    # timing loop too).  FORCE_ASAP is kept as an escape hatch.
    if os.environ.get("FORCE_ASAP") and reps == 1:
        os.environ["TILE_SCHEDULER"] = "asap"
    else:
        os.environ.pop("TILE_SCHEDULER", None)
    import concourse.bacc as bacc
    import concourse.bass as bass
    import concourse.mybir as mybir
    import concourse.tile as tile
    import contextlib

    Ls = list(key)
    F, K, XW, R, xoff, yoff, ekoff, koff = _shapes(Ls)
    NB = len(F)
    FT = yoff[-1]          # total query rows (== xt width, valid rows only)
    EKW = ekoff[-1]        # total ek width
    KT = koff[-1]          # total key tiles

    F32 = mybir.dt.float32
    F16 = mybir.dt.float16
    BF16 = mybir.dt.bfloat16
    F8 = mybir.dt.float8e4

    nc = bacc.Bacc("TRN2", target_bir_lowering=False, debug=False)

    xt_d = nc.dram_tensor("xt", [128, NCC, FT], F16, kind="ExternalInput")
    wq_d = nc.dram_tensor("wq", [128, NCC, 128], F16, kind="ExternalInput")
    wk_d = nc.dram_tensor("wk", [128, NCC, 128], F16, kind="ExternalInput")
    wv_d = nc.dram_tensor("wv", [128, NCC, 128], F16, kind="ExternalInput")
    pw_d = nc.dram_tensor("pw", [128, C], BF16, kind="ExternalInput")
    ek_d = nc.dram_tensor("ek", [128, EKW], BF16, kind="ExternalInput")
    y_d = nc.dram_tensor("y", [FT, C], BF16, kind="ExternalOutput")

    def vkey(b, kt):
        return min(128, F[b] - kt * 128)   # valid keys in tile kt

    # process order: big batches first (deep weave), smallest key-count last
    # (short un-weavable tail)
    border = sorted(range(NB), key=lambda b: (-K[b], -F[b]))

    with tile.TileContext(nc) as tc:
        with (
            tc.For_i(0, reps, 1) if reps > 1 else contextlib.nullcontext(),
            tc.tile_pool(name="yspool", bufs=3) as yspool,
            tc.tile_pool(name="work", bufs=3) as work,
            tc.tile_pool(name="ptpool", bufs=3) as ptpool,
            tc.tile_pool(name="singles", bufs=1) as singles,
            tc.tile_pool(name="psA", bufs=2, space="PSUM") as psA,
            tc.tile_pool(name="psST", bufs=2, space="PSUM") as psST,
            tc.tile_pool(name="psOT", bufs=1, space="PSUM") as psOT,
        ):
            # ---- resident SBUF tensors (xt last: address-layout sensitive) --
            nbias_sb = singles.tile([128, 1], F32, tag="nbias")
            qt_sb = singles.tile([128, FT], F16, tag="qt")
            kt_sb = singles.tile([128, FT], F16, tag="kt")
            otn_sb = singles.tile([128, FT], BF16, tag="otn")
            vp_sb = singles.tile([128, KT, 4, 64], BF16, tag="vp")
            ek_sb = singles.tile([128, EKW], BF16, tag="ek")
            wq_sb = singles.tile([128, NCC, 128], F16, tag="wq")
            wk_sb = singles.tile([128, NCC, 128], F16, tag="wk")
            wv_sb = singles.tile([128, NCC, 128], F16, tag="wv")
            pw_sb = singles.tile([128, C], BF16, tag="pw")
            xt_sb = singles.tile([128, NCC, FT], F16, tag="xt")

            nc.vector.memset(nbias_sb, -44.0)

            # ---- input DMAs ------------------------------------------------
            # SP carries the critical path + late batches; Pool (SWDGE) takes
            # the second batch early, before its multiply stream starts; ACT
            # only pw (tiny).  DVE stays DMA-free.
            def xdma(b, eng):
                eng.dma_start(
                    out=xt_sb[:, :, xoff[b] : xoff[b] + F[b]],
                    in_=xt_d.ap()[:, :, xoff[b] : xoff[b] + F[b]],
                )

            def ekdma(b, eng):
                eng.dma_start(
                    out=ek_sb[:, ekoff[b] : ekoff[b + 1]],
                    in_=ek_d.ap()[:, ekoff[b] : ekoff[b + 1]],
                )

            o0, o1, o2, o3 = border
            nc.sync.dma_start(out=wq_sb, in_=wq_d.ap())
            # first batch's x split in chunks so its first QKV matmuls can
            # start as soon as the first chunk lands
            for qo, ql in _chunks(F[o0]):
                nc.sync.dma_start(
                    out=xt_sb[:, :, xoff[o0] + qo : xoff[o0] + qo + ql],
                    in_=xt_d.ap()[:, :, xoff[o0] + qo : xoff[o0] + qo + ql],
                )
            nc.sync.dma_start(out=wk_sb, in_=wk_d.ap())
            nc.sync.dma_start(out=wv_sb, in_=wv_d.ap())
            xdma(o2, eng=nc.sync)
            ekdma(o2, eng=nc.sync)
            xdma(o3, eng=nc.sync)
            ekdma(o3, eng=nc.sync)
            ekdma(o0, eng=nc.gpsimd)
            xdma(o1, eng=nc.gpsimd)
            ekdma(o1, eng=nc.gpsimd)
            nc.scalar.dma_start(out=pw_sb, in_=pw_d.ap())

            # ---- phase thunks ---------------------------------------------
            DMAONLY = bool(os.environ.get("V7DMAONLY"))
            NOY = bool(os.environ.get("V7NOY"))
            EXPSPLIT = bool(os.environ.get("V7EXPSPLIT"))
            VPSIMPLE = bool(os.environ.get("V7VPSIMPLE"))
            NOPOOLMUL = bool(os.environ.get("V7NOPOOLMUL"))
            ekmul_i = [0]  # DVE/Pool split counter for the et*ek multiplies

            def qkv_thunks(b):
                ts = []
                for w_sb, t_sb in ((wq_sb, qt_sb), (wk_sb, kt_sb)):
                    for qo, ql in _chunks(F[b]):
                        def t(w_sb=w_sb, t_sb=t_sb, qo=qo, ql=ql, b=b):
                            if DMAONLY:
                                return
                            ps = psA.tile([128, 512], F32, tag="ps")
                            for cc in range(NCC):
                                nc.tensor.matmul(
                                    ps[:, 0:ql],
                                    w_sb[:, cc],
                                    xt_sb[:, cc, xoff[b] + qo : xoff[b] + qo + ql],
                                    start=(cc == 0),
                                    stop=(cc == NCC - 1),
                                )
                            nc.scalar.copy(
                                out=t_sb[:, xoff[b] + qo : xoff[b] + qo + ql],
                                in_=ps[:, 0:ql],
                            )
                        ts.append(t)
                # V: pack up to 4 full key tiles per PSUM bank; the final
                # partial tile (vk < 128) gets its own allocation so the
                # group copy never reads uninitialized rows.
                groups = []
                nfull = K[b] if F[b] % 128 == 0 else K[b] - 1
                kt0 = 0
                while kt0 < nfull:
                    gn = min(4, nfull - kt0)
                    groups.append((kt0, gn))
                    kt0 += gn
                if nfull < K[b]:
                    groups.append((nfull, 1))
                for kt0, gn in groups:
                    def t(kt0=kt0, gn=gn, b=b):
                        if DMAONLY:
                            return
                        vkl = vkey(b, kt0 + gn - 1)   # last tile may be short
                        ps = psA.tile([128, 4, 2, 64], F32, tag="ps", name="vps")
                        for g in range(gn):
                            kt = kt0 + g
                            vk = vkey(b, kt)
                            for cc in range(NCC):
                                nc.tensor.matmul(
                                    ps[0:vk, g, :, :],
                                    xt_sb[:, cc, xoff[b] + kt * 128 : xoff[b] + kt * 128 + vk],
                                    wv_sb[:, cc],
                                    start=(cc == 0),
                                    stop=(cc == NCC - 1),
                                    skip_group_check=True,
                                )
                        kk = koff[b] + kt0
                        # vp blocks per key tile: [ch_e | ones | ones | ch_o];
                        # OT_e reads blocks 0:2 = [channels | denom ones],
                        # OT_o reads 2:4.  One strided copy per group fills
                        # all channel blocks.
                        if VPSIMPLE:
                            for g in range(gn):
                                vk = vkey(b, kt0 + g)
                                nc.vector.tensor_copy(
                                    out=vp_sb[0:vk, kk + g, 0, :],
                                    in_=ps[0:vk, g, 0, :],
                                )
                                nc.vector.tensor_copy(
                                    out=vp_sb[0:vk, kk + g, 3, :],
                                    in_=ps[0:vk, g, 1, :],
                                )
                                nc.gpsimd.memset(vp_sb[:, kk + g, 1:3, :], 1.0)
                        else:
                            if gn == 1:
                                nc.vector.tensor_copy(
                                    out=vp_sb[0:vkl, kk, 0:4:3, :],
                                    in_=ps[0:vkl, 0, :, :],
                                )
                            else:
                                nc.vector.tensor_copy(
                                    out=vp_sb[:, kk : kk + gn, 0:4:3, :],
                                    in_=ps[:, 0:gn, :, :],
                                )
                            nc.gpsimd.memset(vp_sb[:, kk : kk + gn, 1:3, :], 1.0)
                    ts.append(t)
                return ts

            def attn_thunks(b):
                ts = []
                for qo, ql in _chunks(F[b]):
                    ot_pair = []
                    st_q = []

                    def emit_st(kt, qo=qo, ql=ql, b=b):
                        if DMAONLY:
                            return
                        vk = vkey(b, kt)
                        # both heads' logits in one 2-bank tile: head h in
                        # bank h, so a single strided [vk, 2, ql] AP covers
                        # both for the exp without touching the bank gaps
                        stp = psST.tile([128, 2, 512], F32, tag="st")
                        for lo, hi, par in ((0, 64, 0), (64, 128, 1)):
                            nc.tensor.matmul(
                                stp[0:vk, par, 0:ql],
                                kt_sb[lo:hi, xoff[b] + kt * 128 : xoff[b] + kt * 128 + vk],
                                qt_sb[lo:hi, yoff[b] + qo : yoff[b] + qo + ql],
                                start=True,
                                stop=True,
                                skip_group_check=True,
                            )
                        st_q.append((kt, stp))

                    def emit_tail(qo=qo, ql=ql, b=b, ot_pair=ot_pair):
                        if DMAONLY:
                            return
                        kt, stp = st_q.pop(0)
                        vk = vkey(b, kt)
                        if kt == 0:
                            ot_pair.append(
                                psOT.tile([128, 2, 512], F32, tag="ot", name="otp")
                            )
                        otp = ot_pair[0]
                        e0 = ekoff[b] + kt * F[b] + qo
                        et = work.tile([128, 2, 512], BF16, tag="et")
                        if EXPSPLIT:
                            for par in (0, 1):
                                nc.scalar.activation(
                                    out=et[0:vk, par, 0:ql],
                                    in_=stp[0:vk, par, 0:ql],
                                    func=mybir.ActivationFunctionType.Exp,
                                    bias=nbias_sb[0:vk, :],
                                )
                        else:
                            nc.scalar.activation(
                                out=et[0:vk, 0:2, 0:ql],
                                in_=stp[0:vk, 0:2, 0:ql],
                                func=mybir.ActivationFunctionType.Exp,
                                bias=nbias_sb[0:vk, :],
                            )
                        pt = ptpool.tile([128, 2, 512], BF16, tag="pt")
                        # 1-partition contractions fault the device; for a
                        # single valid key pad the contraction with a zeroed
                        # second row (vp rows are zeroed too so 0*0 stays 0)
                        vkc = max(vk, 2)
                        if vk < 2:
                            nc.vector.memset(pt[vk:2, :, 0:ql], 0.0)
                            nc.vector.memset(
                                vp_sb[vk:2, koff[b] + kt, :, :], 0.0
                            )
                        for par in (0, 1):
                            # every 4th multiply goes to Pool to relieve DVE
                            eng = nc.vector if NOPOOLMUL else (
                                nc.gpsimd if ekmul_i[0] % 4 == 3 else nc.vector)
                            ekmul_i[0] += 1
                            eng.tensor_mul(
                                pt[0:vk, par, 0:ql],
                                et[0:vk, par, 0:ql],
                                ek_sb[0:vk, e0 : e0 + ql],
                            )
                            nc.tensor.matmul(
                                otp[:, par, 0:ql],
                                vp_sb[0:vkc, koff[b] + kt, 2 * par : 2 * par + 2, :],
                                pt[0:vkc, par, 0:ql],
                                start=(kt == 0),
                                stop=(kt == K[b] - 1),
                            )

                    # ST-lead pipeline: PE runs kt+1's ST matmuls while kt's
                    # exp/mult chain drains
                    for kt in range(K[b]):
                        def t(kt=kt, es=emit_st, etl=emit_tail, kb=K[b]):
                            es(kt)
                            if kt >= 1:
                                etl()
                            if kt == kb - 1:
                                etl()
                        ts.append(t)

                    def t(qo=qo, ql=ql, b=b, ot_pair=ot_pair):
                        if DMAONLY:
                            return
                        otp = ot_pair[0]
                        qa = yoff[b] + qo
                        rb = work.tile([128, 512], BF16, tag="rb")
                        with nc.allow_low_precision(reason="bf16 denom recip"):
                            nc.vector.reciprocal(
                                out=rb[0:64, 0:ql], in_=otp[64:128, 0, 0:ql]
                            )
                            nc.vector.reciprocal(
                                out=rb[64:128, 0:ql], in_=otp[0:64, 1, 0:ql]
                            )
                        nc.vector.tensor_mul(
                            otn_sb[0:64, qa : qa + ql],
                            otp[0:64, 0, 0:ql], rb[0:64, 0:ql],
                        )
                        nc.vector.tensor_mul(
                            otn_sb[64:128, qa : qa + ql],
                            otp[64:128, 1, 0:ql], rb[64:128, 0:ql],
                        )
                    ts.append(t)
                return ts

            def proj_thunks(b):
                ts = []
                pi = [0]
                for rt in range(R[b]):
                    def t(rt=rt, b=b):
                        if DMAONLY or NOY:
                            return
                        rows = min(128, F[b] - rt * 128)
                        r0 = yoff[b] + rt * 128
                        ys = yspool.tile([128, C], BF16, tag="ys")
                        for oc in range(2):
                            pj = psA.tile([128, 512], F32, tag="ps", name="pj")
                            nc.tensor.matmul(
                                pj[0:rows, :],
                                otn_sb[:, r0 : r0 + rows],
                                pw_sb[:, oc * 512 : (oc + 1) * 512],
                                start=True,
                                stop=True,
                            )
                            if pi[0] % 2 == 0:
                                nc.scalar.copy(
                                    out=ys[0:rows, oc * 512 : (oc + 1) * 512],
                                    in_=pj[0:rows, :],
                                )
                            else:
                                nc.vector.tensor_copy(
                                    out=ys[0:rows, oc * 512 : (oc + 1) * 512],
                                    in_=pj[0:rows, :],
                                )
                            pi[0] += 1
                        nc.sync.dma_start(
                            out=y_d.ap()[r0 : r0 + rows, :],
                            in_=ys[0:rows, :],
                        )
                    ts.append(t)
                return ts

            # ---- 3-stage pipelined emission -------------------------------
            # qkv(b_{i+1}) and proj(b_{i-1}) weave into attn(b_i)'s stream.
            A = [qkv_thunks(b) for b in border]
            Bt = [attn_thunks(b) for b in border]
            Ct = [proj_thunks(b) for b in border]

            if os.environ.get("V7SERIAL"):
                for i in range(NB):
                    for t in A[i] + Bt[i] + Ct[i]:
                        t()
            else:
                for t in A[0]:
                    t()
                for i in range(NB):
                    lists = [Bt[i]]
                    if i + 1 < NB:
                        lists.append(A[i + 1])
                    if i - 1 >= 0:
                        lists.append(Ct[i - 1])
                    while any(lists):
                        for ls in lists:
                            if ls:
                                ls.pop(0)()
                for t in Ct[NB - 1]:
                    t()

    nc.compile()
    return nc


def _pad_for(L):
    # retained name for test.py compatibility: returns the program cache key
    return tuple(int(l) for l in L)


def _prep_inputs(key, x, K, n1, n2, qkv_w, qkv_b, proj_w):
    import ml_dtypes

    Ls = list(key)
    F, Kt, XW, R, xoff, yoff, ekoff, koff = _shapes(Ls)
    FT, EKW = yoff[-1], ekoff[-1]
    scale = np.float32(Dh**-0.5)
    assert not np.any(qkv_b), "nonzero qkv_b not supported by this kernel"
    bf16 = ml_dtypes.bfloat16
    f8 = ml_dtypes.float8_e4m3fn

    # xt: [128, FT, 8] fp16 (valid rows only, batches concatenated; per-batch
    # slices are fully contiguous per partition for single-descriptor DMAs)
    X_all = np.empty((FT, C), dtype=np.float16)
    for b in range(B):
        X_all[xoff[b] : xoff[b] + F[b]] = x[b, : F[b]]
    xt = np.ascontiguousarray(
        X_all.T.reshape(NCC, 128, FT).transpose(1, 0, 2)
    )

    # ek: [128, EKW] bf16 multiplicative exp(K), zeros on padded/masked keys
    ekp = np.zeros((128, EKW), dtype=bf16)
    for b in range(B):
        E = np.zeros((XW[b], F[b]), dtype=np.float32)
        E[: F[b], :] = np.exp(K[b, : F[b], : F[b]].astype(np.float32)).T
        ekp[:, ekoff[b] : ekoff[b + 1]] = (
            E.reshape(Kt[b], 128, F[b]).transpose(1, 0, 2).reshape(128, -1)
        )

    def wslice(w, j):
        # w rows [128j:128j+128] of [C, C]; -> [128 p(cc), 8 cc, 128 m]
        ws = np.ascontiguousarray(
            w[128 * j : 128 * (j + 1), :].T.reshape(NCC, 128, 128).transpose(1, 0, 2)
        )
        return ws

    in_maps = []
    for j in range(8):
        wq = wslice(qkv_w[0 * C : 1 * C] * scale, j).astype(np.float16)
        wk = wslice(qkv_w[1 * C : 2 * C], j).astype(np.float16)
        wv = wslice(qkv_w[2 * C : 3 * C], j).astype(np.float16)
        pw = np.ascontiguousarray(
            proj_w[:, 128 * j : 128 * (j + 1)].T
        ).astype(bf16)
        in_maps.append(
            {"xt": xt, "wq": wq, "wk": wk, "wv": wv, "pw": pw, "ek": ekp}
        )
    L = np.asarray(Ls, dtype=np.int32)
    return in_maps, L


def run_device(inputs, trace=False):
    """Compile (cached), run on 8 cores, return (BassKernelResults, L)."""
    from concourse import bass_utils

    x = np.asarray(inputs["x"], dtype=np.float32)
    K = np.asarray(inputs["K"], dtype=np.float32)
    n1 = np.asarray(inputs["n1"])
    n2 = np.asarray(inputs["n2"])
    L = (n1.astype(np.int64) * n2.astype(np.int64)).astype(np.int32)
    key = _pad_for(L)
    if ("nc", key) not in _CACHE:
        _CACHE[("nc", key)] = _build_program(key)
    nc = _CACHE[("nc", key)]

    in_maps, L = _prep_inputs(
        key, x, K, n1, n2,
        np.asarray(inputs["qkv_w"], dtype=np.float32),
        np.asarray(inputs["qkv_b"], dtype=np.float32),
        np.asarray(inputs["proj_w"], dtype=np.float32),
    )
    res = bass_utils.run_bass_kernel_spmd(
        nc, in_maps, core_ids=list(range(8)), trace=trace
    )
    return res, L


def kernel(**inputs):
    x = np.asarray(inputs["x"], dtype=np.float32)
    qkv_w = np.asarray(inputs["qkv_w"], dtype=np.float32)
    qkv_b = np.asarray(inputs["qkv_b"], dtype=np.float32)
    proj_w = np.asarray(inputs["proj_w"], dtype=np.float32)
    proj_b = np.asarray(inputs["proj_b"], dtype=np.float32)

    res, L = run_device(inputs)
    Fs, _, _, _, _, yoff, _, _ = _shapes(L)

    ysum = np.zeros((yoff[-1], C), dtype=np.float32)
    for r in res.results:
        ysum += np.asarray(r["y"], dtype=np.float32)
    ysum += proj_b

    out = np.empty((B, N, C), dtype=np.float32)
    for b in range(B):
        Lb = int(L[b])
        out[b, :Lb] = ysum[yoff[b] : yoff[b] + Lb]
        # fully-masked rows: exactly uniform softmax -> mean of V
        vbar = x[b].mean(axis=0) @ qkv_w[2 * C : 3 * C, :].T + qkv_b[2 * C : 3 * C]
        out[b, Lb:] = vbar @ proj_w.T + proj_b
    return out
